# revision 1
# baseline (speedup 1.0000x reference)
"""GCN (2-layer graph convolution, symmetric norm) on 8 TRN2 NeuronCores.

Node-sharded graph/data-parallel, 3 launches (per sharding hint):
 - Phase A (node-sharded MLP, transposed dataflow): each core computes
   h1s = (lrelu(x@W1+b1)@W2+b2) * rsqrt(max(deg_s,1)) for its 12500-node
   range (fp8e4m3 table out). Host supplies x pre-transposed, so zero
   on-device transposes:  h1T = W1^T @ xT (lhsT=W1), Lrelu+bias on the
   scalar engine, then h2 = h1T^T @ W2 (lhsT=h1T) lands row-major.
 - Halo exchange between launches is host-mediated: the host gathers
   h1s[senders] / h2s[senders] into per-core, receiver-sorted edge-row
   streams (partition-major layout), so each launch only does full-
   bandwidth sequential DMA - no on-device random access.
 - Phase B (edge-sharded by receiver): per 128-receiver block, segment-sum
   the streamed fp8 rows with one-hot matmuls accumulating the TRANSPOSED
   aggregate (lhsT=g, rhs=onehot), so the Lrelu'd result feeds the decode
   matmul directly as lhsT with no transpose. inv_r/bias/inv_s algebra is
   folded into a rank-1 bias matmul (sqrt(deg_r) x bd) and one per-
   partition scale (inv_r*inv_s). One-hot matrices for 49 blocks x ksub
   are built in ONE DVE op per supertile via 3D broadcast is_equal.
 - Phase C: same aggregation over bf16 h2s edge rows (40-wide), then a
   fused softmax: Exp activation with scale=inv_r, bias=-inv_r*max,
   accum_out for the denominator.
"""

import numpy as np
import ml_dtypes

N = 100000
E = 600000
D = 128
C = 40
NCORES = 8
NS = N // NCORES          # 12500 nodes per core
P = 128
NB = (NS + P - 1) // P    # 98 blocks per core
NPAD = NB * P             # 12544
SUPA = 14                 # phase-A supertile (blocks per DMA); NB % SUPA == 0
SUPB = 7                  # phase-B/C supertile (blocks per stream tile)
NSUP = NB // SUPB         # 14

BF16 = ml_dtypes.bfloat16
F8 = ml_dtypes.float8_e4m3


def _ctx():
    from concourse import bass, bacc, mybir, tile
    return bass, bacc, mybir, tile


def _loop(tc, repeat):
    import contextlib
    if repeat > 1:
        return tc.For_i(0, repeat)
    return contextlib.nullcontext()


def _build_phase_a(repeat=1):
    bass, bacc, mybir, tile = _ctx()
    f32 = mybir.dt.float32
    bf16 = mybir.dt.bfloat16
    f8 = mybir.dt.float8e4
    nc = bacc.Bacc("TRN2", target_bir_lowering=False, debug=False)
    xT_ext = nc.declare_dram_parameter("xT", [D, NPAD], bf16, isOutput=False)
    w1_ext = nc.declare_dram_parameter("W1", [D, D], bf16, isOutput=False)
    b1_ext = nc.declare_dram_parameter("b1c", [D, 1], f32, isOutput=False)
    w2_ext = nc.declare_dram_parameter("W2", [D, D], bf16, isOutput=False)
    b2_ext = nc.declare_dram_parameter("b2r", [1, D], bf16, isOutput=False)
    ones_ext = nc.declare_dram_parameter("ones_row", [1, P], bf16, isOutput=False)
    invs_ext = nc.declare_dram_parameter("invs_pc", [P, NB], f32, isOutput=False)
    out_ext = nc.declare_dram_parameter("h1s_pm", [P, NB, D], f8, isOutput=True)

    with tile.TileContext(nc) as tc:
        with (
            tc.tile_pool(name="const", bufs=1) as cp,
            tc.tile_pool(name="xin", bufs=3) as xp,
            tc.tile_pool(name="mid", bufs=6) as sp,
            tc.tile_pool(name="outp", bufs=3) as op,
            tc.tile_pool(name="psum1", bufs=3, space="PSUM") as pp1,
            tc.tile_pool(name="psum2", bufs=3, space="PSUM") as pp2,
        ):
            w1 = cp.tile([D, D], dtype=bf16)
            nc.sync.dma_start(out=w1[:], in_=w1_ext[:])
            b1c = cp.tile([D, 1], dtype=f32)
            nc.sync.dma_start(out=b1c[:], in_=b1_ext[:])
            w2 = cp.tile([D, D], dtype=bf16)
            nc.sync.dma_start(out=w2[:], in_=w2_ext[:])
            b2r = cp.tile([1, D], dtype=bf16)
            nc.sync.dma_start(out=b2r[:], in_=b2_ext[:])
            ones = cp.tile([1, P], dtype=bf16)
            nc.sync.dma_start(out=ones[:], in_=ones_ext[:])
            invs = cp.tile([P, NB], dtype=f32)
            nc.sync.dma_start(out=invs[:], in_=invs_ext[:])

            with _loop(tc, repeat):
                for s in range(NB // SUPA):
                    xt = xp.tile([D, SUPA * P], dtype=bf16)
                    nc.sync.dma_start(
                        out=xt[:], in_=xT_ext[:, s * SUPA * P:(s + 1) * SUPA * P]
                    )
                    o_super = op.tile([P, SUPA, D], dtype=f8)
                    # mm1 + lrelu batched over groups of 4 blocks (one PSUM bank)
                    h1Ts = []
                    for g0 in range(0, SUPA, 4):
                        gw = min(4, SUPA - g0)
                        ps1 = pp1.tile([D, 4 * P], dtype=f32, space="PSUM")
                        nc.tensor.matmul(
                            out=ps1[:, :gw * P], lhsT=w1[:],
                            rhs=xt[:, g0 * P:(g0 + gw) * P],
                            start=True, stop=True,
                        )
                        h1T = sp.tile([D, 4 * P], dtype=bf16)
                        nc.scalar.activation(
                            out=h1T[:, :gw * P], in_=ps1[:, :gw * P],
                            func=mybir.ActivationFunctionType.Lrelu,
                            bias=b1c[:, 0:1], alpha=0.01,
                        )
                        h1Ts.append(h1T)
                    for j in range(SUPA):
                        b = s * SUPA + j
                        h1T = h1Ts[j // 4]
                        jj = j % 4
                        ps2 = pp2.tile([P, D], dtype=f32, space="PSUM")
                        nc.tensor.matmul(
                            out=ps2[:], lhsT=h1T[:, jj * P:(jj + 1) * P], rhs=w2[:],
                            start=True, stop=False,
                        )
                        nc.tensor.matmul(
                            out=ps2[:], lhsT=ones[:], rhs=b2r[:], start=False, stop=True
                        )
                        nc.vector.tensor_tensor(
                            out=o_super[:, j, :], in0=ps2[:],
                            in1=invs[:, b:b + 1].to_broadcast([P, D]),
                            op=mybir.AluOpType.mult,
                        )
                    nc.gpsimd.dma_start(
                        out=out_ext[:, s * SUPA:(s + 1) * SUPA, :], in_=o_super[:]
                    )
    nc.finalize()
    return nc


def _build_phase_b(ksub, repeat=1):
    bass, bacc, mybir, tile = _ctx()
    f32 = mybir.dt.float32
    bf16 = mybir.dt.bfloat16
    f8 = mybir.dt.float8e4
    KS = SUPB * ksub              # stream columns per supertile
    nc = bacc.Bacc("TRN2", target_bir_lowering=False, debug=False)
    ed_ext = nc.declare_dram_parameter("edB", [NSUP, P, KS, D], f8, isOutput=False)
    lrecv_ext = nc.declare_dram_parameter("lrecvT", [P, NB * ksub], bf16, isOutput=False)
    iota_ext = nc.declare_dram_parameter("iota", [P, P], bf16, isOutput=False)
    wd_ext = nc.declare_dram_parameter("Wd", [D, C], bf16, isOutput=False)
    bd_ext = nc.declare_dram_parameter("bd", [1, C], bf16, isOutput=False)
    sq_ext = nc.declare_dram_parameter("sq_row", [1, NPAD], bf16, isOutput=False)
    prod_ext = nc.declare_dram_parameter("prod_pc", [P, NB], f32, isOutput=False)
    out_ext = nc.declare_dram_parameter("h2s_pm", [P, NB, C], bf16, isOutput=True)

    with tile.TileContext(nc) as tc:
        with (
            tc.tile_pool(name="const", bufs=1) as cp,
            tc.tile_pool(name="gath", bufs=3) as gp,
            tc.tile_pool(name="oneh", bufs=3) as mp,
            tc.tile_pool(name="mid", bufs=6) as sp,
            tc.tile_pool(name="outp", bufs=3) as op,
            tc.tile_pool(name="psuma", bufs=3, space="PSUM") as ppa,
            tc.tile_pool(name="psumo", bufs=3, space="PSUM") as ppo,
        ):
            iota = cp.tile([P, P], dtype=bf16)
            nc.sync.dma_start(out=iota[:], in_=iota_ext[:])
            wd = cp.tile([D, C], dtype=bf16)
            nc.sync.dma_start(out=wd[:], in_=wd_ext[:])
            bd = cp.tile([1, C], dtype=bf16)
            nc.sync.dma_start(out=bd[:], in_=bd_ext[:])
            sq = cp.tile([1, NPAD], dtype=bf16)
            nc.sync.dma_start(out=sq[:], in_=sq_ext[:])
            prod = cp.tile([P, NB], dtype=f32)
            nc.sync.dma_start(out=prod[:], in_=prod_ext[:])
            lrc = cp.tile([P, NB * ksub], dtype=bf16)
            nc.sync.dma_start(out=lrc[:], in_=lrecv_ext[:])

            with _loop(tc, repeat):
                for s in range(NSUP):
                    g = gp.tile([P, KS, D], dtype=f8)
                    nc.sync.dma_start(out=g[:], in_=ed_ext[s, :, :, :])
                    # one-hot in [p, k, r] layout: contiguous r-slices so the
                    # matmul's MOVING operand is unit-stride (strided moving
                    # operands are slow on HW; strided stationary is fine)
                    m = mp.tile([P, KS, P], dtype=bf16)
                    nc.vector.tensor_tensor(
                        out=m[:],
                        in0=lrc[:, s * KS:(s + 1) * KS].unsqueeze(2)
                            .to_broadcast([P, KS, P]),
                        in1=iota[:].unsqueeze(1).to_broadcast([P, KS, P]),
                        op=mybir.AluOpType.is_equal,
                    )
                    o_super = op.tile([P, SUPB, C], dtype=bf16)
                    for j in range(SUPB):
                        b = s * SUPB + j
                        r0 = b * P
                        ps_aggT = ppa.tile([D, P], dtype=f32, space="PSUM")
                        for k in range(ksub):
                            col = j * ksub + k
                            nc.tensor.matmul(
                                out=ps_aggT[:], lhsT=g[:, col, :],
                                rhs=m[:, col, :],
                                start=(k == 0), stop=(k == ksub - 1),
                            )
                        hT = sp.tile([D, P], dtype=bf16)
                        nc.scalar.activation(
                            out=hT[:], in_=ps_aggT[:],
                            func=mybir.ActivationFunctionType.Lrelu, alpha=0.01,
                        )
                        ps_o = ppo.tile([P, C], dtype=f32, space="PSUM")
                        nc.tensor.matmul(
                            out=ps_o[:], lhsT=hT[:], rhs=wd[:], start=True, stop=False
                        )
                        nc.tensor.matmul(
                            out=ps_o[:], lhsT=sq[0:1, r0:r0 + P], rhs=bd[:],
                            start=False, stop=True,
                        )
                        nc.scalar.activation(
                            out=o_super[:, j, :], in_=ps_o[:],
                            func=mybir.ActivationFunctionType.Copy,
                            scale=prod[:, b:b + 1],
                        )
                    nc.gpsimd.dma_start(
                        out=out_ext[:, s * SUPB:(s + 1) * SUPB, :], in_=o_super[:]
                    )
    nc.finalize()
    return nc


def _build_phase_c(ksub, repeat=1):
    bass, bacc, mybir, tile = _ctx()
    f32 = mybir.dt.float32
    bf16 = mybir.dt.bfloat16
    KS = SUPB * ksub
    nc = bacc.Bacc("TRN2", target_bir_lowering=False, debug=False)
    ed_ext = nc.declare_dram_parameter("edC", [NSUP, P, KS * C], bf16, isOutput=False)
    lrecv_ext = nc.declare_dram_parameter("lrecvT", [P, NB * ksub], bf16, isOutput=False)
    iota_ext = nc.declare_dram_parameter("iota_rep", [P, P, KS], bf16, isOutput=False)
    invr_ext = nc.declare_dram_parameter("invr_pc", [P, NB], f32, isOutput=False)
    out_ext = nc.declare_dram_parameter("res_pm", [P, NB, C], f32, isOutput=True)

    with tile.TileContext(nc) as tc:
        with (
            tc.tile_pool(name="const", bufs=1) as cp,
            tc.tile_pool(name="gath", bufs=3) as gp,
            tc.tile_pool(name="oneh", bufs=3) as mp,
            tc.tile_pool(name="mid", bufs=8) as sp,
            tc.tile_pool(name="outp", bufs=3) as op,
            tc.tile_pool(name="psuma", bufs=4, space="PSUM") as ppa,
        ):
            iota = cp.tile([P, P, KS], dtype=bf16)
            nc.sync.dma_start(out=iota[:], in_=iota_ext[:])
            invr = cp.tile([P, NB], dtype=f32)
            nc.sync.dma_start(out=invr[:], in_=invr_ext[:])
            lrc = cp.tile([P, NB * ksub], dtype=bf16)
            nc.sync.dma_start(out=lrc[:], in_=lrecv_ext[:])

            with _loop(tc, repeat):
                for s in range(NSUP):
                    g = gp.tile([P, KS * C], dtype=bf16)
                    nc.sync.dma_start(out=g[:], in_=ed_ext[s, :, :])
                    m = mp.tile([P, P, KS], dtype=bf16)
                    nc.vector.tensor_tensor(
                        out=m[:],
                        in0=lrc[:, s * KS:(s + 1) * KS].unsqueeze(1)
                            .to_broadcast([P, P, KS]),
                        in1=iota[:],
                        op=mybir.AluOpType.is_equal,
                    )
                    o_super = op.tile([P, SUPB, C], dtype=f32)
                    for j in range(SUPB):
                        b = s * SUPB + j
                        ps = ppa.tile([P, C], dtype=f32, space="PSUM")
                        for k in range(ksub):
                            col = j * ksub + k
                            nc.tensor.matmul(
                                out=ps[:], lhsT=m[:, :, col],
                                rhs=g[:, col * C:(col + 1) * C],
                                start=(k == 0), stop=(k == ksub - 1),
                            )
                        # logits are bounded well below exp-overflow range, so
                        # no max-subtraction; denominator via accum_out and
                        # the divide on the (otherwise idle) Pool engine.
                        ex = sp.tile([P, C], dtype=f32)
                        den = sp.tile([P, 1], dtype=f32)
                        nc.scalar.activation(
                            out=ex[:], in_=ps[:],
                            func=mybir.ActivationFunctionType.Exp,
                            scale=invr[:, b:b + 1],
                            accum_out=den[:, 0:1],
                        )
                        rec = sp.tile([P, 1], dtype=f32)
                        nc.vector.reciprocal(rec[:], den[:])
                        nc.scalar.activation(
                            out=o_super[:, j, :], in_=ex[:],
                            func=mybir.ActivationFunctionType.Copy,
                            scale=rec[:, 0:1],
                        )
                    nc.scalar.dma_start(
                        out=out_ext[:, s * SUPB:(s + 1) * SUPB, :], in_=o_super[:]
                    )
    nc.finalize()
    return nc


_EXEC_TIMES = []
_LAST = {}


def _run(nc, in_maps):
    from concourse.bass_utils import run_bass_kernel_spmd
    res = run_bass_kernel_spmd(nc, in_maps, core_ids=list(range(NCORES)))
    if res.exec_time_ns is not None:
        _EXEC_TIMES.append(res.exec_time_ns)
    return res.results


def _prep(x, senders, receivers, W1, b1, W2, b2, Wd, bd):
    """Host-side index preprocessing and per-core input assembly."""
    deg_s = np.bincount(senders, minlength=N).astype(np.float32)
    deg_r = np.bincount(receivers, minlength=N).astype(np.float32)
    inv_s = (1.0 / np.sqrt(np.maximum(deg_s, 1.0))).astype(np.float32)
    inv_r = (1.0 / np.sqrt(np.maximum(deg_r, 1.0))).astype(np.float32)
    sq_r = np.sqrt(np.maximum(deg_r, 1.0)).astype(np.float32)

    order = np.argsort(receivers, kind="stable")
    rs = receivers[order]
    ss = senders[order]

    bounds = []
    for c in range(NCORES):
        for b in range(NB):
            lo = c * NS + b * P
            hi = c * NS + min((b + 1) * P, NS)
            bounds.append((lo, hi))
    lows = np.searchsorted(rs, [lo for lo, _ in bounds], side="left")
    highs = np.searchsorted(rs, [hi for _, hi in bounds], side="left")
    counts = highs - lows
    kmax = int(counts.max())
    ksub = max(1, (kmax + P - 1) // P)
    K = ksub * P

    # slot tables: eidx[c, b, p, k] = global edge (into ss) or -1 for pad
    eidx = np.full((NCORES, NB, P, ksub), -1, np.int64)
    lrecvT = np.full((NCORES, NB, P, ksub), -1.0, np.float32)
    lbuf = np.empty(K, np.float32)
    ebuf = np.empty(K, np.int64)
    for c in range(NCORES):
        for b in range(NB):
            i = c * NB + b
            lo, hi, m = lows[i], highs[i], counts[i]
            ebuf[:] = -1
            lbuf[:] = -1.0
            ebuf[:m] = np.arange(lo, hi)
            lbuf[:m] = (rs[lo:hi] - (c * NS + b * P)).astype(np.float32)
            eidx[c, b] = ebuf.reshape(ksub, P).T
            lrecvT[c, b] = lbuf.reshape(ksub, P).T
    # sender per slot (pad -> sender 0, killed by -1 lrecv one-hot)
    send_slot = np.where(eidx >= 0, ss[np.clip(eidx, 0, E - 1)], 0)  # [NC,NB,P,ksub]
    # device lrecv layout [P, NB*ksub]: col b*ksub+k
    lrecv_dev = np.ascontiguousarray(
        lrecvT.transpose(0, 2, 1, 3).reshape(NCORES, P, NB * ksub)
    ).astype(BF16)
    # stream slot layout [NSUP, P, SUPB*ksub] of sender ids, for host gather
    send_stream = np.ascontiguousarray(
        send_slot.reshape(NCORES, NSUP, SUPB, P, ksub)
        .transpose(0, 1, 3, 2, 4)
        .reshape(NCORES, NSUP, P, SUPB * ksub)
    )

    KS = SUPB * ksub
    iota = np.tile(np.arange(P, dtype=np.float32)[None, :], (P, 1)).astype(BF16)
    iota_rep = np.ascontiguousarray(np.broadcast_to(
        np.arange(P, dtype=np.float32)[None, :, None], (P, P, KS))).astype(BF16)
    ones_row = np.ones((1, P), np.float32).astype(BF16)

    def pcol(v, fill=1.0):  # [N] -> [NCORES, P, NB]
        out = np.full((NCORES, NPAD), fill, np.float32)
        out[:, :NS] = v.reshape(NCORES, NS)
        return np.ascontiguousarray(out.reshape(NCORES, NB, P).transpose(0, 2, 1))

    invs_pc = pcol(inv_s)
    invr_pc = pcol(inv_r)
    ninvr_pc = pcol(-inv_r, fill=-1.0)
    prod_pc = pcol(inv_r * inv_s)
    sq_row = np.ones((NCORES, 1, NPAD), np.float32)
    sq_row[:, 0, :NS] = sq_r.reshape(NCORES, NS)
    sq_row = sq_row.astype(BF16)

    xT = np.zeros((NCORES, D, NPAD), np.float32)
    for c in range(NCORES):
        xT[c, :, :NS] = x[c * NS:(c + 1) * NS].T
    xT = xT.astype(BF16)

    return dict(
        ksub=ksub, xT=xT, send_stream=send_stream, lrecv_dev=lrecv_dev,
        iota=iota, iota_rep=iota_rep, ones_row=ones_row,
        invs_pc=invs_pc, invr_pc=invr_pc,
        prod_pc=prod_pc, sq_row=sq_row,
        W1=W1.astype(BF16), b1c=b1.reshape(D, 1).astype(np.float32),
        W2=W2.astype(BF16), b2r=b2.reshape(1, D).astype(BF16),
        Wd=Wd.astype(BF16), bd=bd.reshape(1, C).astype(BF16),
    )


def _pm_to_nodes(pm):
    """[NCORES][P, NB, W] core outputs -> [N, W] node-major."""
    w = pm[0].shape[-1]
    out = np.empty((NCORES, NS, w), pm[0].dtype)
    for c in range(NCORES):
        out[c] = np.ascontiguousarray(pm[c].transpose(1, 0, 2)).reshape(NPAD, w)[:NS]
    return out.reshape(N, w)


def kernel(x, senders, receivers, W1, b1, W2, b2, Wd, bd):
    x = np.asarray(x, np.float32)
    senders = np.asarray(senders, np.int32)
    receivers = np.asarray(receivers, np.int32)
    pr = _prep(x, senders, receivers,
               np.asarray(W1, np.float32), np.asarray(b1, np.float32),
               np.asarray(W2, np.float32), np.asarray(b2, np.float32),
               np.asarray(Wd, np.float32), np.asarray(bd, np.float32))
    ksub = pr["ksub"]
    KS = SUPB * ksub

    # ---- phase A ----
    nc_a = _build_phase_a()
    maps_a = [
        {"xT": pr["xT"][c], "W1": pr["W1"], "b1c": pr["b1c"], "W2": pr["W2"],
         "b2r": pr["b2r"], "ones_row": pr["ones_row"], "invs_pc": pr["invs_pc"][c]}
        for c in range(NCORES)
    ]
    _LAST["A"] = maps_a
    res_a = _run(nc_a, maps_a)
    h1s = _pm_to_nodes([np.asarray(r["h1s_pm"]) for r in res_a])  # [N, D] fp8

    # ---- host halo exchange: edge-row streams for phase B ----
    sstr = pr["send_stream"]  # [NC, NSUP, P, KS]
    edB = [np.ascontiguousarray(
        h1s[sstr[c].reshape(-1)].reshape(NSUP, P, KS, D)) for c in range(NCORES)]

    nc_b = _build_phase_b(ksub)
    maps_b = [
        {"edB": edB[c], "lrecvT": pr["lrecv_dev"][c], "iota": pr["iota"],
         "Wd": pr["Wd"], "bd": pr["bd"], "sq_row": pr["sq_row"][c],
         "prod_pc": pr["prod_pc"][c]}
        for c in range(NCORES)
    ]
    _LAST["B"] = maps_b
    res_b = _run(nc_b, maps_b)
    h2s = _pm_to_nodes([np.asarray(r["h2s_pm"]) for r in res_b])  # [N, C] bf16

    # ---- host halo exchange for phase C ----
    edC = [np.ascontiguousarray(
        h2s[sstr[c].reshape(-1)].reshape(NSUP, P, KS * C)) for c in range(NCORES)]

    nc_c = _build_phase_c(ksub)
    maps_c = [
        {"edC": edC[c], "lrecvT": pr["lrecv_dev"][c], "iota_rep": pr["iota_rep"],
         "invr_pc": pr["invr_pc"][c]}
        for c in range(NCORES)
    ]
    _LAST["C"] = maps_c
    res_c = _run(nc_c, maps_c)
    out = _pm_to_nodes([np.asarray(r["res_pm"]) for r in res_c])  # [N, C] f32
    _LAST["ksub"] = ksub
    return np.ascontiguousarray(out).astype(np.float32)



# revision 4
# speedup vs baseline: 7.2756x; 7.2756x over previous
"""GCN (2-layer graph convolution, symmetric norm) on 8 TRN2 NeuronCores.

Node-sharded graph/data-parallel, 3 launches. Key ideas vs the naive
one-hot-matmul segment-sum:

 - Nodes are permuted into 64-node "bins" balanced by receiver degree
   (greedy heap packing). Every bin's edge count fits kp*128 slots with
   kp = ceil(max_bin_degree/128) = 3 for this graph (0.35% padding).
   Two adjacent bins form a 128-receiver "pair" block; all downstream
   dense work (lrelu, decode matmul, scaling, softmax) runs at 128-wide
   pair granularity.
 - One-hot matrices are built in the [slot_p, r(64), group] layout whose
   operands are all packed 2-byte APs, hitting the DVE 2x perf mode, and
   at 64-wide receiver windows (half the elements of 128-wide). The
   aggregation matmuls read strided moving/stationary slices of it
   (free in the cost model) and write 64-wide column halves of the
   pair's PSUM accumulator.
 - inv_s (sender norm) of conv1 is folded into x on the host
   (lrelu(a*x) = a*lrelu(x) for a>0; bias terms enter as rank-1 matmuls
   only when nonzero). inv_r*inv_s of conv1->conv2 is applied per pair
   by the otherwise-idle GpSimd engine. Softmax: Pool pre-scales by
   inv_r, ACT does one batched Exp per supertile, DVE reduces the
   denominators, Pool applies the reciprocal.
 - Host mediates halo exchange between launches: gathers h1/h2 rows
   into per-core receiver-sorted slot streams (fp8 / bf16), so each
   launch only does full-bandwidth sequential DMA.
"""

import heapq

import numpy as np
import ml_dtypes

N = 100000
E = 600000
D = 128
C = 40
NCORES = 8
P = 128
BIN = 64
NBINS = (N + BIN - 1) // BIN + (64 - 1)  # round bins so NBINS % (2*NCORES) == 0
# choose NBINS: need NBINS*BIN >= N, NBINS % (2*NCORES) == 0 for whole pairs
NBINS = ((N + BIN - 1) // BIN + 2 * NCORES - 1) // (2 * NCORES) * (2 * NCORES)
NPADG = NBINS * BIN               # 100352 padded global nodes
NBIN_CORE = NBINS // NCORES       # 196
NPAIR = NBIN_CORE // 2            # 98 pair-blocks per core
NS = NBIN_CORE * BIN              # 12544 nodes per core
SUPP = 7                          # pairs per supertile (phases B/C)
NSUP = NPAIR // SUPP              # 14
SUPA = 7                          # 128-col blocks per phase-A supertile
NSUP_A = NS // (SUPA * P)         # 14

BF16 = ml_dtypes.bfloat16
F8 = ml_dtypes.float8_e4m3


def _ctx():
    from concourse import bass, bacc, mybir, tile
    return bass, bacc, mybir, tile


# --------------------------------------------------------------------------
# device kernels
# --------------------------------------------------------------------------

def _build_phase_a(has_b1, has_b2):
    bass, bacc, mybir, tile = _ctx()
    f32 = mybir.dt.float32
    bf16 = mybir.dt.bfloat16
    f8 = mybir.dt.float8e4
    W = SUPA * P  # 896 columns per supertile
    nc = bacc.Bacc("TRN2", target_bir_lowering=False, debug=False)
    xs_ext = nc.declare_dram_parameter("xsT", [D, NS], bf16, isOutput=False)
    w1_ext = nc.declare_dram_parameter("W1", [D, D], bf16, isOutput=False)
    w2_ext = nc.declare_dram_parameter("W2", [D, D], bf16, isOutput=False)
    if has_b1 or has_b2:
        invs_ext = nc.declare_dram_parameter("invs_row", [1, NS], bf16, isOutput=False)
    if has_b1:
        b1_ext = nc.declare_dram_parameter("b1_row", [1, D], bf16, isOutput=False)
    if has_b2:
        b2_ext = nc.declare_dram_parameter("b2_row", [1, D], bf16, isOutput=False)
    out_ext = nc.declare_dram_parameter("h1T_pm", [D, NS], f8, isOutput=True)

    with tile.TileContext(nc) as tc:
        with (
            tc.tile_pool(name="const", bufs=1) as cp,
            tc.tile_pool(name="xin", bufs=3) as xp,
            tc.tile_pool(name="mid", bufs=2) as sp,
            tc.tile_pool(name="outp", bufs=3) as op,
            tc.tile_pool(name="psum1", bufs=2, space="PSUM") as pp1,
            tc.tile_pool(name="psum2", bufs=2, space="PSUM") as pp2,
        ):
            w1 = cp.tile([D, D], dtype=bf16, name="w1")
            nc.sync.dma_start(out=w1[:], in_=w1_ext[:])
            w2 = cp.tile([D, D], dtype=bf16, name="w2")
            nc.sync.dma_start(out=w2[:], in_=w2_ext[:])
            if has_b1 or has_b2:
                invs = cp.tile([1, NS], dtype=bf16, name="invs")
                nc.sync.dma_start(out=invs[:], in_=invs_ext[:])
            if has_b1:
                b1r = cp.tile([1, D], dtype=bf16, name="b1r")
                nc.sync.dma_start(out=b1r[:], in_=b1_ext[:])
            if has_b2:
                b2r = cp.tile([1, D], dtype=bf16, name="b2r")
                nc.sync.dma_start(out=b2r[:], in_=b2_ext[:])

            for s in range(NSUP_A):
                c0 = s * W
                xt = xp.tile([D, W], dtype=bf16, name="xt")
                nc.sync.dma_start(out=xt[:], in_=xs_ext[:, c0:c0 + W])
                ps1 = pp1.tile([D, W], dtype=f32, space="PSUM", name="ps1")
                nc.tensor.matmul(out=ps1[:, 0:512], lhsT=w1[:], rhs=xt[:, 0:512],
                                 start=True, stop=not has_b1)
                nc.tensor.matmul(out=ps1[:, 512:W], lhsT=w1[:], rhs=xt[:, 512:W],
                                 start=True, stop=not has_b1)
                if has_b1:
                    nc.tensor.matmul(out=ps1[:, 0:512], lhsT=b1r[:],
                                     rhs=invs[0:1, c0:c0 + 512],
                                     start=False, stop=True)
                    nc.tensor.matmul(out=ps1[:, 512:W], lhsT=b1r[:],
                                     rhs=invs[0:1, c0 + 512:c0 + W],
                                     start=False, stop=True)
                h1 = sp.tile([D, W], dtype=bf16, name="h1")
                nc.scalar.activation(
                    out=h1[:], in_=ps1[:],
                    func=mybir.ActivationFunctionType.Lrelu, alpha=0.01,
                )
                ps2 = pp2.tile([D, W], dtype=f32, space="PSUM", name="ps2")
                nc.tensor.matmul(out=ps2[:, 0:512], lhsT=w2[:], rhs=h1[:, 0:512],
                                 start=True, stop=not has_b2)
                nc.tensor.matmul(out=ps2[:, 512:W], lhsT=w2[:], rhs=h1[:, 512:W],
                                 start=True, stop=not has_b2)
                if has_b2:
                    nc.tensor.matmul(out=ps2[:, 0:512], lhsT=b2r[:],
                                     rhs=invs[0:1, c0:c0 + 512],
                                     start=False, stop=True)
                    nc.tensor.matmul(out=ps2[:, 512:W], lhsT=b2r[:],
                                     rhs=invs[0:1, c0 + 512:c0 + W],
                                     start=False, stop=True)
                o = op.tile([D, W], dtype=f8, name="o")
                nc.vector.tensor_copy(out=o[:], in_=ps2[:])
                nc.scalar.dma_start(out=out_ext[:, c0:c0 + W], in_=o[:])
    nc.finalize()
    return nc


def _build_phase_b(kp, has_bd):
    bass, bacc, mybir, tile = _ctx()
    f32 = mybir.dt.float32
    bf16 = mybir.dt.bfloat16
    f8 = mybir.dt.float8e4
    GP = 2 * kp                  # slot groups per pair
    KS = SUPP * GP               # slot groups per supertile
    nc = bacc.Bacc("TRN2", target_bir_lowering=False, debug=False)
    ed_ext = nc.declare_dram_parameter("edB", [NSUP, P, KS, D], f8, isOutput=False)
    lrc_ext = nc.declare_dram_parameter("lrcT", [P, NPAIR * GP], bf16, isOutput=False)
    iota_ext = nc.declare_dram_parameter("iota_rep", [P, BIN, KS], bf16, isOutput=False)
    wd_ext = nc.declare_dram_parameter("Wd", [D, C], bf16, isOutput=False)
    prod_ext = nc.declare_dram_parameter("prod_pc", [P, NPAIR], f32, isOutput=False)
    if has_bd:
        sqr_ext = nc.declare_dram_parameter("sqr_row", [1, NPAIR * P], bf16,
                                            isOutput=False)
        bd_ext = nc.declare_dram_parameter("bd_row", [1, C], bf16, isOutput=False)
    out_ext = nc.declare_dram_parameter("h2s_pm", [P, NPAIR, C], bf16, isOutput=True)

    with tile.TileContext(nc) as tc:
        with (
            tc.tile_pool(name="const", bufs=1) as cp,
            tc.tile_pool(name="gath", bufs=3) as gp,
            tc.tile_pool(name="oneh", bufs=3) as mp,
            tc.tile_pool(name="mid", bufs=2) as sp,
            tc.tile_pool(name="outp", bufs=3) as op,
            tc.tile_pool(name="psuma", bufs=2, space="PSUM") as ppa,
            tc.tile_pool(name="psumo", bufs=2, space="PSUM") as ppo,
        ):
            iota = cp.tile([P, BIN, KS], dtype=bf16, name="iota")
            nc.sync.dma_start(out=iota[:], in_=iota_ext[:])
            wd = cp.tile([D, C], dtype=bf16, name="wd")
            nc.sync.dma_start(out=wd[:], in_=wd_ext[:])
            prod = cp.tile([P, NPAIR], dtype=f32, name="prod")
            nc.sync.dma_start(out=prod[:], in_=prod_ext[:])
            lrc = cp.tile([P, NPAIR * GP], dtype=bf16, name="lrc")
            nc.sync.dma_start(out=lrc[:], in_=lrc_ext[:])
            if has_bd:
                sqr = cp.tile([1, NPAIR * P], dtype=bf16, name="sqr")
                nc.sync.dma_start(out=sqr[:], in_=sqr_ext[:])
                bdr = cp.tile([1, C], dtype=bf16, name="bdr")
                nc.sync.dma_start(out=bdr[:], in_=bd_ext[:])

            for s in range(NSUP):
                g = gp.tile([P, KS, D], dtype=f8, name="g")
                nc.sync.dma_start(out=g[:], in_=ed_ext[s, :, :, :])
                m = mp.tile([P, BIN, KS], dtype=bf16, name="m")
                nc.vector.tensor_tensor(
                    out=m[:],
                    in0=lrc[:, s * KS:(s + 1) * KS].unsqueeze(1)
                        .to_broadcast([P, BIN, KS]),
                    in1=iota[:],
                    op=mybir.AluOpType.is_equal,
                )
                psA = ppa.tile([D, SUPP, P], dtype=f32, space="PSUM", name="psA")
                for j in range(SUPP):
                    for gi in range(GP):
                        col = j * GP + gi
                        off = BIN * (gi // kp)
                        nc.tensor.matmul(
                            out=psA[:, j, off:off + BIN],
                            lhsT=g[:, col, :], rhs=m[:, :, col],
                            start=(gi % kp == 0), stop=(gi % kp == kp - 1),
                        )
                hT = sp.tile([D, SUPP, P], dtype=bf16, name="hT")
                nc.scalar.activation(
                    out=hT[:], in_=psA[:],
                    func=mybir.ActivationFunctionType.Lrelu, alpha=0.01,
                )
                psO = ppo.tile([P, SUPP, C], dtype=f32, space="PSUM", name="psO")
                for j in range(SUPP):
                    nc.tensor.matmul(out=psO[:, j, :], lhsT=hT[:, j, :], rhs=wd[:],
                                     start=True, stop=not has_bd)
                    if has_bd:
                        q = s * SUPP + j
                        nc.tensor.matmul(
                            out=psO[:, j, :], lhsT=sqr[0:1, q * P:(q + 1) * P],
                            rhs=bdr[:], start=False, stop=True,
                        )
                # GpSimd can't read PSUM: drain via one batched ACT copy,
                # then the per-node prod scale runs on the idle Pool engine.
                hdec = sp.tile([P, SUPP, C], dtype=f32, name="hdec")
                nc.scalar.activation(
                    out=hdec[:], in_=psO[:],
                    func=mybir.ActivationFunctionType.Copy,
                )
                o = op.tile([P, SUPP, C], dtype=bf16, name="o")
                nc.gpsimd.tensor_tensor(
                    out=o[:], in0=hdec[:],
                    in1=prod[:, s * SUPP:(s + 1) * SUPP].unsqueeze(2)
                        .to_broadcast([P, SUPP, C]),
                    op=mybir.AluOpType.mult,
                )
                nc.scalar.dma_start(
                    out=out_ext[:, s * SUPP:(s + 1) * SUPP, :], in_=o[:]
                )
    nc.finalize()
    return nc


def _build_phase_c(kp):
    bass, bacc, mybir, tile = _ctx()
    f32 = mybir.dt.float32
    bf16 = mybir.dt.bfloat16
    GP = 2 * kp
    KS = SUPP * GP
    nc = bacc.Bacc("TRN2", target_bir_lowering=False, debug=False)
    ed_ext = nc.declare_dram_parameter("edC", [NSUP, P, KS * C], bf16, isOutput=False)
    lrc_ext = nc.declare_dram_parameter("lrcT", [P, NPAIR * GP], bf16, isOutput=False)
    iota_ext = nc.declare_dram_parameter("iota_rep", [P, BIN, KS], bf16, isOutput=False)
    invr_ext = nc.declare_dram_parameter("invr_pc", [P, NPAIR], f32, isOutput=False)
    out_ext = nc.declare_dram_parameter("res_pm", [P, NPAIR, C], bf16, isOutput=True)

    with tile.TileContext(nc) as tc:
        with (
            tc.tile_pool(name="const", bufs=1) as cp,
            tc.tile_pool(name="gath", bufs=3) as gp,
            tc.tile_pool(name="oneh", bufs=3) as mp,
            tc.tile_pool(name="mid", bufs=3) as sp,
            tc.tile_pool(name="outp", bufs=3) as op,
            tc.tile_pool(name="psumc", bufs=2, space="PSUM") as ppc,
        ):
            iota = cp.tile([P, BIN, KS], dtype=bf16, name="iota")
            nc.sync.dma_start(out=iota[:], in_=iota_ext[:])
            invr = cp.tile([P, NPAIR], dtype=f32, name="invr")
            nc.sync.dma_start(out=invr[:], in_=invr_ext[:])
            lrc = cp.tile([P, NPAIR * GP], dtype=bf16, name="lrc")
            nc.sync.dma_start(out=lrc[:], in_=lrc_ext[:])

            for s in range(NSUP):
                g = gp.tile([P, KS * C], dtype=bf16, name="g")
                nc.sync.dma_start(out=g[:], in_=ed_ext[s, :, :])
                m = mp.tile([P, BIN, KS], dtype=bf16, name="m")
                nc.vector.tensor_tensor(
                    out=m[:],
                    in0=lrc[:, s * KS:(s + 1) * KS].unsqueeze(1)
                        .to_broadcast([P, BIN, KS]),
                    in1=iota[:],
                    op=mybir.AluOpType.is_equal,
                )
                psC = ppc.tile([P, SUPP, C], dtype=f32, space="PSUM", name="psC")
                for j in range(SUPP):
                    for gi in range(GP):
                        col = j * GP + gi
                        off = BIN * (gi // kp)
                        nc.tensor.matmul(
                            out=psC[off:off + BIN, j, :],
                            lhsT=m[:, :, col], rhs=g[:, col * C:(col + 1) * C],
                            start=(gi % kp == 0), stop=(gi % kp == kp - 1),
                        )
                # GpSimd can't read PSUM: ACT drains the aggregate, Pool
                # applies the inv_r temperature from SBUF, ACT exponentiates.
                agg = sp.tile([P, SUPP, C], dtype=f32, name="agg")
                nc.scalar.activation(
                    out=agg[:], in_=psC[:],
                    func=mybir.ActivationFunctionType.Copy,
                )
                sc = sp.tile([P, SUPP, C], dtype=f32, name="sc")
                nc.gpsimd.tensor_tensor(
                    out=sc[:], in0=agg[:],
                    in1=invr[:, s * SUPP:(s + 1) * SUPP].unsqueeze(2)
                        .to_broadcast([P, SUPP, C]),
                    op=mybir.AluOpType.mult,
                )
                ex = sp.tile([P, SUPP, C], dtype=f32, name="ex")
                nc.scalar.activation(
                    out=ex[:], in_=sc[:],
                    func=mybir.ActivationFunctionType.Exp,
                )
                den = sp.tile([P, SUPP], dtype=f32, name="den")
                nc.vector.tensor_reduce(
                    out=den[:], in_=ex[:], axis=mybir.AxisListType.X,
                    op=mybir.AluOpType.add,
                )
                rec = sp.tile([P, SUPP], dtype=f32, name="rec")
                nc.vector.reciprocal(rec[:], den[:])
                o = op.tile([P, SUPP, C], dtype=bf16, name="o")
                nc.gpsimd.tensor_tensor(
                    out=o[:], in0=ex[:],
                    in1=rec[:].unsqueeze(2).to_broadcast([P, SUPP, C]),
                    op=mybir.AluOpType.mult,
                )
                nc.scalar.dma_start(
                    out=out_ext[:, s * SUPP:(s + 1) * SUPP, :], in_=o[:]
                )
    nc.finalize()
    return nc


# --------------------------------------------------------------------------
# host side
# --------------------------------------------------------------------------

_EXEC_TIMES = []
_LAST = {}


def _run(nc, in_maps):
    from concourse.bass_utils import run_bass_kernel_spmd
    res = run_bass_kernel_spmd(nc, in_maps, core_ids=list(range(NCORES)))
    if res.exec_time_ns is not None:
        _EXEC_TIMES.append(res.exec_time_ns)
    return res.results


def _balance_bins(deg_r):
    """Greedy heap packing of nodes into 64-node bins balanced by receiver
    degree. Returns new-node-id per old node (new id = bin*64 + pos) and the
    max per-bin degree sum."""
    order = np.argsort(-deg_r, kind="stable")
    heap = [(0, b) for b in range(NBINS)]
    heapq.heapify(heap)
    loads = np.zeros(NBINS, np.int64)
    counts = np.zeros(NBINS, np.int32)
    new_id = np.empty(N, np.int64)
    for node in order:
        dg = int(deg_r[node])
        while True:
            load, b = heapq.heappop(heap)
            if load == loads[b]:
                break
        new_id[node] = b * BIN + counts[b]
        loads[b] += dg
        counts[b] += 1
        if counts[b] < BIN:
            heapq.heappush(heap, (loads[b], b))
    return new_id, int(loads.max())


def _prep(x, senders, receivers, W1, b1, W2, b2, Wd, bd):
    deg_s = np.bincount(senders, minlength=N).astype(np.float32)
    deg_r = np.bincount(receivers, minlength=N).astype(np.float32)
    inv_s = (1.0 / np.sqrt(np.maximum(deg_s, 1.0))).astype(np.float32)
    inv_r = (1.0 / np.sqrt(np.maximum(deg_r, 1.0))).astype(np.float32)
    sq_r = np.sqrt(np.maximum(deg_r, 1.0)).astype(np.float32)

    new_id, max_load = _balance_bins(deg_r.astype(np.int64))
    kp = max(1, (max_load + P - 1) // P)
    GP = 2 * kp
    KS = SUPP * GP
    new2old = np.full(NPADG, -1, np.int64)
    new2old[new_id] = np.arange(N)

    # ---- edge slot assignment ----
    r_new = new_id[receivers]
    s_new = new_id[senders]
    bin_e = (r_new >> 6).astype(np.int64)
    loc_e = (r_new & 63).astype(np.int64)
    order = np.argsort(bin_e, kind="stable")
    counts = np.bincount(bin_e, minlength=NBINS)
    starts = np.concatenate([[0], np.cumsum(counts)[:-1]])
    pos_in_bin = np.arange(E) - starts[bin_e[order]]

    SLOTS = kp * P
    eidx = np.full((NBINS, SLOTS), -1, np.int64)
    eidx[bin_e[order], pos_in_bin] = order
    valid = eidx >= 0
    lrc_slot = np.where(valid, loc_e[np.clip(eidx, 0, E - 1)], -1).astype(np.float32)
    send_slot = np.where(valid, s_new[np.clip(eidx, 0, E - 1)], 0).astype(np.int64)

    # device layouts: [NBINS, kp, P] -> per-core [P, NPAIR, GP] etc.
    # column (q*GP + gi): bin = core_base + 2q + gi//kp, k = gi % kp
    def to_cols(a, fill):
        # a: [NBINS, SLOTS] -> [NCORES, P, NPAIR*GP]
        a = a.reshape(NCORES, NPAIR, 2, kp, P)        # [c, q, parity, k, p]
        a = a.transpose(0, 4, 1, 2, 3)                # [c, p, q, parity, k]
        return np.ascontiguousarray(a.reshape(NCORES, P, NPAIR * GP))

    lrcT = to_cols(lrc_slot, -1.0).astype(BF16)
    send_cols = to_cols(send_slot, 0)                 # [c, P, NPAIR*GP] int64
    send_stream = send_cols.reshape(NCORES, P, NSUP, KS).transpose(0, 2, 1, 3)
    send_stream = np.ascontiguousarray(send_stream)   # [c, NSUP, P, KS]

    iota_rep = np.ascontiguousarray(np.broadcast_to(
        np.arange(BIN, dtype=np.float32)[None, :, None], (P, BIN, KS))).astype(BF16)

    # per-new-node column tables [NCORES, P, NPAIR]
    def pcol(v, fill):
        out = np.full(NPADG, fill, np.float32)
        out[new_id] = v
        return np.ascontiguousarray(
            out.reshape(NCORES, NPAIR, P).transpose(0, 2, 1))

    invr_pc = pcol(inv_r, 1.0)
    prod_pc = pcol(inv_r * inv_s, 1.0)
    sqr_row = np.ones(NPADG, np.float32)
    sqr_row[new_id] = sq_r
    sqr_row = sqr_row.reshape(NCORES, 1, NS).astype(BF16)

    # phase A input: x pre-scaled by inv_s, permuted, transposed per core
    xs = np.zeros((NPADG, D), np.float32)
    xs[new_id] = x * inv_s[:, None]
    xsT = np.ascontiguousarray(
        xs.reshape(NCORES, NS, D).transpose(0, 2, 1)).astype(BF16)
    invs_row = np.zeros(NPADG, np.float32)
    invs_row[new_id] = inv_s
    invs_row = invs_row.reshape(NCORES, 1, NS).astype(BF16)

    return dict(
        kp=kp, GP=GP, KS=KS, new_id=new_id, new2old=new2old,
        lrcT=lrcT, send_stream=send_stream, iota_rep=iota_rep,
        invr_pc=invr_pc, prod_pc=prod_pc, sqr_row=sqr_row,
        xsT=xsT, invs_row=invs_row,
        W1=W1.astype(BF16), W2=W2.astype(BF16), Wd=Wd.astype(BF16),
        b1=b1, b2=b2, bd=bd,
        has_b1=bool(np.any(b1)), has_b2=bool(np.any(b2)),
        has_bd=bool(np.any(bd)),
    )


def kernel(x, senders, receivers, W1, b1, W2, b2, Wd, bd):
    x = np.asarray(x, np.float32)
    senders = np.asarray(senders, np.int32)
    receivers = np.asarray(receivers, np.int32)
    pr = _prep(x, senders, receivers,
               np.asarray(W1, np.float32), np.asarray(b1, np.float32),
               np.asarray(W2, np.float32), np.asarray(b2, np.float32),
               np.asarray(Wd, np.float32), np.asarray(bd, np.float32))
    kp, GP, KS = pr["kp"], pr["GP"], pr["KS"]

    # ---- phase A ----
    nc_a = _build_phase_a(pr["has_b1"], pr["has_b2"])
    maps_a = []
    for c in range(NCORES):
        mA = {"xsT": pr["xsT"][c], "W1": pr["W1"], "W2": pr["W2"]}
        if pr["has_b1"] or pr["has_b2"]:
            mA["invs_row"] = pr["invs_row"][c]
        if pr["has_b1"]:
            mA["b1_row"] = pr["b1"].reshape(1, D).astype(BF16)
        if pr["has_b2"]:
            mA["b2_row"] = pr["b2"].reshape(1, D).astype(BF16)
        maps_a.append(mA)
    _LAST["A"] = (nc_a, maps_a)
    res_a = _run(nc_a, maps_a)
    # h1 rows per global new node [NPADG, D] fp8
    h1_all = np.empty((NPADG, D), F8)
    for c in range(NCORES):
        h1_all[c * NS:(c + 1) * NS] = np.asarray(res_a[c]["h1T_pm"]).T

    # ---- host halo gather for phase B ----
    sstr = pr["send_stream"]  # [c, NSUP, P, KS]
    edB = [np.ascontiguousarray(
        h1_all[sstr[c].reshape(-1)].reshape(NSUP, P, KS, D))
        for c in range(NCORES)]

    nc_b = _build_phase_b(kp, pr["has_bd"])
    maps_b = []
    for c in range(NCORES):
        mB = {"edB": edB[c], "lrcT": pr["lrcT"][c], "iota_rep": pr["iota_rep"],
              "Wd": pr["Wd"], "prod_pc": pr["prod_pc"][c]}
        if pr["has_bd"]:
            mB["sqr_row"] = pr["sqr_row"][c]
            mB["bd_row"] = pr["bd"].reshape(1, C).astype(BF16)
        maps_b.append(mB)
    _LAST["B"] = (nc_b, maps_b)
    res_b = _run(nc_b, maps_b)
    # h2 rows [NPADG, C] bf16 (pair-block partition-major per core)
    h2_all = np.empty((NPADG, C), BF16)
    for c in range(NCORES):
        pm = np.asarray(res_b[c]["h2s_pm"])        # [P, NPAIR, C]
        h2_all[c * NS:(c + 1) * NS] = pm.transpose(1, 0, 2).reshape(NS, C)

    # ---- host halo gather for phase C ----
    edC = [np.ascontiguousarray(
        h2_all[sstr[c].reshape(-1)].reshape(NSUP, P, KS * C))
        for c in range(NCORES)]

    nc_c = _build_phase_c(kp)
    maps_c = [
        {"edC": edC[c], "lrcT": pr["lrcT"][c], "iota_rep": pr["iota_rep"],
         "invr_pc": pr["invr_pc"][c]}
        for c in range(NCORES)
    ]
    _LAST["C"] = (nc_c, maps_c)
    res_c = _run(nc_c, maps_c)
    res_new = np.empty((NPADG, C), np.float32)
    for c in range(NCORES):
        pm = np.asarray(res_c[c]["res_pm"]).astype(np.float32)
        res_new[c * NS:(c + 1) * NS] = pm.transpose(1, 0, 2).reshape(NS, C)
    return np.ascontiguousarray(res_new[pr["new_id"]])
